# revision 26
# baseline (speedup 1.0000x reference)
"""Block-circulant linear layer (y = x @ W^T + bias, W built from 64x64
circulant blocks) on 8 Trainium2 NeuronCores.

Math: per output block j, input block i: y[t,j] = sum_i circ(c[j,i]) @ x[t,i].
Via the convolution theorem this is, for each rfft bin k:
    Yhat[t,j,k] = sum_i Chat[j,i,k] * Xhat[t,i,k]   (complex)
i.e. 33 independent complex [64 x 64] matmuls over the block index, batched
over tokens. The host does the cheap O(T*F*logB) DFTs + layout packing; the
device does the dominant compute — the per-frequency complex matmuls — packed
as 32 real [128x128] @ [128x512] matmuls per core (data-parallel over tokens).

Real/complex packing (per frequency k, contraction over rows r):
    rhs rows r:   [Xr_i (64) ; Xi_i (64)],  cols = tokens
    lhsT[i,    j] =  Cr[j,i]    lhsT[i,    64+j] = Ci[j,i]
    lhsT[64+i, j] = -Ci[j,i]    lhsT[64+i, 64+j] = Cr[j,i]
    out rows:     [Yr_j (64) ; Yi_j (64)]
Bins k=0 and k=32 are purely real (real input DFT), so they share one tile
(kt=0) with a block-diagonal lhsT; kt=1..31 carry bin k = kt.
"""

import numpy as np

_B = 64          # circulant block size
_NBLK = 64       # input/output blocks (4096/64)
_NK = 33         # rfft bins of a 64-point real signal
_NKT = 32        # packed frequency tiles (k0+k32 share tile 0)
_NCORES = 8
_T = 4096        # tokens = 2*2048
_TCORE = _T // _NCORES
_F = 4096

_CACHE = {}


# 4 k-tiles ride in each DMA transfer (512 KB in fp16): free-dim layout (kt%KTB, t)
_KTB = 4
_NG = _NKT // _KTB   # 8 DMA groups

# matmul input precision: "fp32r" (fp32 bits, TF32-grade multiply) or "fp16"
# (half the input DMA bytes, ~4x coarser rounding). Accumulation is fp32 either way.
_IN_PREC = "fp16"
# device->host precision of the frequency-domain result (fp16 halves store bytes;
# values are O(10) so fp16 rounding is ~2e-4 relative)
_OUT_PREC = "fp16"
_NP_IN = {"fp32r": np.float32, "fp16": np.float16}


def _build_cmat(c):
    """c: [J=64, I=64, B=64] float32 -> packed lhsT matrix [128, NKT*128]."""
    fc = np.fft.rfft(np.asarray(c, np.float32), axis=-1)  # [J, I, 33] complex64
    Cr, Ci = fc.real, fc.imag
    cm = np.zeros((_NKT, 128, 128), np.float32)  # [kt, row, col]
    cm[0, 0:64, 0:64] = Cr[:, :, 0].T
    cm[0, 64:128, 64:128] = Cr[:, :, 32].T
    for k in range(1, 32):
        cm[k, 0:64, 0:64] = Cr[:, :, k].T
        cm[k, 64:128, 0:64] = -Ci[:, :, k].T
        cm[k, 0:64, 64:128] = Ci[:, :, k].T
        cm[k, 64:128, 64:128] = Cr[:, :, k].T
    # device layout: [128 partitions, kt*128 + col]
    out = np.ascontiguousarray(cm.transpose(1, 0, 2)).reshape(128, _NKT * 128)
    return out.astype(_NP_IN[_IN_PREC])


def _build_xk(x):
    """x: [2, 2048, 4096] float32 -> packed rhs [NKT, 128, T]."""
    xb = np.asarray(x, np.float32).reshape(_T, _NBLK, _B)
    fx = np.fft.rfft(xb, axis=-1)            # [T, I, 33] complex64
    R = fx.real.transpose(2, 1, 0)           # [33, I, T]
    Im = fx.imag.transpose(2, 1, 0)
    XKf = np.empty((_NKT, 128, _T), np.float32)
    XKf[0, 0:64] = R[0]
    XKf[0, 64:128] = R[32]
    XKf[1:32, 0:64] = R[1:32]
    XKf[1:32, 64:128] = Im[1:32]
    return XKf


def _unpack_y(YKf, bias):
    """YKf: [NKT, 128, T] device output -> y [2, 2048, 4096] float32."""
    re = np.zeros((_NK, _NBLK, _T), np.float32)
    im = np.zeros((_NK, _NBLK, _T), np.float32)
    re[0] = YKf[0, 0:64]
    re[32] = YKf[0, 64:128]
    re[1:32] = YKf[1:32, 0:64]
    im[1:32] = YKf[1:32, 64:128]
    Yf = (re + 1j * im).transpose(2, 1, 0)   # [T, J, 33]
    yb = np.fft.irfft(Yf, n=_B, axis=-1).astype(np.float32)  # [T, J, B]
    y = yb.reshape(_T, _F) + np.asarray(bias, np.float32)
    return np.ascontiguousarray(y.reshape(2, _T // 2, _F))


def _build_device():
    import concourse.bacc as bacc
    import concourse.mybir as mybir
    import concourse.tile as tile

    f32 = mybir.dt.float32
    # float32r: same fp32 bits, but the PE streams 1 column/cycle instead of
    # fp32's 4 (cost: reduced multiply precision; accumulation stays fp32).
    mmdt = {
        "fp32r": mybir.dt.float32r,
        "fp16": mybir.dt.float16,
    }[_IN_PREC]
    outdt = {"fp32": f32, "fp16": mybir.dt.float16}[_OUT_PREC]
    nc = bacc.Bacc("TRN2", target_bir_lowering=False, debug=False)
    xk = nc.dram_tensor("xk", [_NG, 128, _KTB * _TCORE], mmdt, kind="ExternalInput")
    cm = nc.dram_tensor("cm", [128, _NKT * 128], mmdt, kind="ExternalInput")
    yk = nc.dram_tensor("yk", [_NG, 128, _KTB * _TCORE], outdt, kind="ExternalOutput")

    with tile.TileContext(nc) as tc:
        with (
            tc.tile_pool(name="cpool", bufs=1) as cpool,
            tc.tile_pool(name="xpool", bufs=1) as xpool,
            tc.tile_pool(name="ypool", bufs=4) as ypool,
            tc.tile_pool(name="pp", bufs=3, space="PSUM") as pp,
        ):
            # SP-ring order: cm0, xk0, xk1, then remaining cm chunks, then the
            # rest of xk — first matmul's deps land first, transfers stay big
            cts = [
                cpool.tile([128, _KTB * 128], mmdt, tag=f"cw{g}", name=f"cw{g}")
                for g in range(_NG)
            ]
            xts = [
                xpool.tile([128, _KTB * _TCORE], mmdt, tag=f"x{g}", name=f"x{g}")
                for g in range(_NG)
            ]

            def load_cm(g):
                nc.sync.dma_start(
                    out=cts[g][:],
                    in_=cm[:, g * _KTB * 128:(g + 1) * _KTB * 128],
                )

            load_cm(0)
            nc.sync.dma_start(out=xts[0][:], in_=xk[0])
            nc.sync.dma_start(out=xts[1][:], in_=xk[1])
            for g in range(1, _NG):
                load_cm(g)
            for g in range(2, _NG):
                nc.sync.dma_start(out=xts[g][:], in_=xk[g])

            for g in range(_NG):
                xt = xts[g]
                yt = ypool.tile([128, _KTB * _TCORE], outdt)
                for h in range(_KTB // 2):
                    # 2-bank PSUM tile: two matmuls, DVE and ACT each copy one
                    ps = pp.tile([128, 2 * _TCORE], f32)
                    for jj in range(2):
                        j = h * 2 + jj
                        nc.tensor.matmul(
                            ps[:, jj * _TCORE:(jj + 1) * _TCORE],
                            lhsT=cts[g][:, j * 128:(j + 1) * 128],
                            rhs=xt[:, j * _TCORE:(j + 1) * _TCORE],
                            start=True,
                            stop=True,
                        )
                    ybase = h * 2 * _TCORE
                    nc.vector.tensor_copy(
                        yt[:, ybase:ybase + _TCORE], ps[:, 0:_TCORE]
                    )
                    nc.scalar.copy(
                        yt[:, ybase + _TCORE:ybase + 2 * _TCORE],
                        ps[:, _TCORE:2 * _TCORE],
                    )
                # stores on the ACT HWDGE ring
                nc.scalar.dma_start(out=yk[g], in_=yt[:])
    nc.compile()
    return nc


def _execute(in_maps, **kwargs):
    from concourse.bass_utils import run_bass_kernel_spmd

    if "nc" not in _CACHE:
        _CACHE["nc"] = _build_device()
    return run_bass_kernel_spmd(
        _CACHE["nc"], in_maps, core_ids=list(range(_NCORES)), **kwargs
    )


def _make_in_maps(x, c):
    XKf = _build_xk(x)
    cmd = _build_cmat(c)
    maps = []
    for m in range(_NCORES):
        xkm = XKf[:, :, m * _TCORE:(m + 1) * _TCORE]  # [NKT, 128, TCORE]
        xkm = (
            xkm.reshape(_NG, _KTB, 128, _TCORE)
            .transpose(0, 2, 1, 3)
            .reshape(_NG, 128, _KTB * _TCORE)
        )
        maps.append(
            {"xk": np.ascontiguousarray(xkm).astype(_NP_IN[_IN_PREC]), "cm": cmd}
        )
    return maps


def _gather_yk(results):
    """Per-core yk [NG, 128, KTB*TCORE] -> full [NKT, 128, T]."""
    per_core = []
    for r in results:
        ykm = np.asarray(r["yk"]).reshape(_NG, 128, _KTB, _TCORE)
        per_core.append(ykm.transpose(0, 2, 1, 3).reshape(_NKT, 128, _TCORE))
    return np.concatenate(per_core, axis=2)


def kernel(x, c, bias, **_kwargs):
    in_maps = _make_in_maps(x, c)
    bkr = _execute(in_maps)
    return _unpack_y(_gather_yk(bkr.results), bias)


# revision 28
# speedup vs baseline: 1.0163x; 1.0163x over previous
"""Block-circulant linear layer (y = x @ W^T + bias, W built from 64x64
circulant blocks) on 8 Trainium2 NeuronCores.

Math: per output block j, input block i: y[t,j] = sum_i circ(c[j,i]) @ x[t,i].
Via the convolution theorem this is, for each rfft bin k:
    Yhat[t,j,k] = sum_i Chat[j,i,k] * Xhat[t,i,k]   (complex)
i.e. 33 independent complex [64 x 64] matmuls over the block index, batched
over tokens. The host does the cheap O(T*F*logB) DFTs + layout packing; the
device does the dominant compute — the per-frequency complex matmuls — packed
as 32 real [128x128] @ [128x512] matmuls per core (data-parallel over tokens).

Real/complex packing (per frequency k, contraction over rows r):
    rhs rows r:   [Xr_i (64) ; Xi_i (64)],  cols = tokens
    lhsT[i,    j] =  Cr[j,i]    lhsT[i,    64+j] = Ci[j,i]
    lhsT[64+i, j] = -Ci[j,i]    lhsT[64+i, 64+j] = Cr[j,i]
    out rows:     [Yr_j (64) ; Yi_j (64)]
Bins k=0 and k=32 are purely real (real input DFT), so they share one tile
(kt=0) with a block-diagonal lhsT; kt=1..31 carry bin k = kt.
"""

import numpy as np

_B = 64          # circulant block size
_NBLK = 64       # input/output blocks (4096/64)
_NK = 33         # rfft bins of a 64-point real signal
_NKT = 32        # packed frequency tiles (k0+k32 share tile 0)
_NCORES = 8
_T = 4096        # tokens = 2*2048
_TCORE = _T // _NCORES
_F = 4096

_CACHE = {}


# 4 k-tiles ride in each DMA transfer (512 KB in fp16): free-dim layout (kt%KTB, t)
_KTB = 4
_NG = _NKT // _KTB   # 8 DMA groups

# matmul input precision: "fp32r" (fp32 bits, TF32-grade multiply) or "fp16"
# (half the input DMA bytes, ~4x coarser rounding). Accumulation is fp32 either way.
_IN_PREC = "fp16"
# device->host precision of the frequency-domain result (fp16 halves store bytes;
# values are O(10) so fp16 rounding is ~2e-4 relative)
_OUT_PREC = "fp16"
_NP_IN = {"fp32r": np.float32, "fp16": np.float16}


def _build_cmat(c):
    """c: [J=64, I=64, B=64] float32 -> packed lhsT matrix [128, NKT*128]."""
    fc = np.fft.rfft(np.asarray(c, np.float32), axis=-1)  # [J, I, 33] complex64
    Cr, Ci = fc.real, fc.imag
    cm = np.zeros((_NKT, 128, 128), np.float32)  # [kt, row, col]
    cm[0, 0:64, 0:64] = Cr[:, :, 0].T
    cm[0, 64:128, 64:128] = Cr[:, :, 32].T
    for k in range(1, 32):
        cm[k, 0:64, 0:64] = Cr[:, :, k].T
        cm[k, 64:128, 0:64] = -Ci[:, :, k].T
        cm[k, 0:64, 64:128] = Ci[:, :, k].T
        cm[k, 64:128, 64:128] = Cr[:, :, k].T
    # device layout: [128 partitions, kt*128 + col]
    out = np.ascontiguousarray(cm.transpose(1, 0, 2)).reshape(128, _NKT * 128)
    return out.astype(_NP_IN[_IN_PREC])


def _build_xk(x):
    """x: [2, 2048, 4096] float32 -> packed rhs [NKT, 128, T]."""
    xb = np.asarray(x, np.float32).reshape(_T, _NBLK, _B)
    fx = np.fft.rfft(xb, axis=-1)            # [T, I, 33] complex64
    R = fx.real.transpose(2, 1, 0)           # [33, I, T]
    Im = fx.imag.transpose(2, 1, 0)
    XKf = np.empty((_NKT, 128, _T), np.float32)
    XKf[0, 0:64] = R[0]
    XKf[0, 64:128] = R[32]
    XKf[1:32, 0:64] = R[1:32]
    XKf[1:32, 64:128] = Im[1:32]
    return XKf


def _unpack_y(YKf, bias):
    """YKf: [NKT, 128, T] device output -> y [2, 2048, 4096] float32."""
    re = np.zeros((_NK, _NBLK, _T), np.float32)
    im = np.zeros((_NK, _NBLK, _T), np.float32)
    re[0] = YKf[0, 0:64]
    re[32] = YKf[0, 64:128]
    re[1:32] = YKf[1:32, 0:64]
    im[1:32] = YKf[1:32, 64:128]
    Yf = (re + 1j * im).transpose(2, 1, 0)   # [T, J, 33]
    yb = np.fft.irfft(Yf, n=_B, axis=-1).astype(np.float32)  # [T, J, B]
    y = yb.reshape(_T, _F) + np.asarray(bias, np.float32)
    return np.ascontiguousarray(y.reshape(2, _T // 2, _F))


def _build_device():
    import concourse.bacc as bacc
    import concourse.mybir as mybir
    import concourse.tile as tile

    f32 = mybir.dt.float32
    # float32r: same fp32 bits, but the PE streams 1 column/cycle instead of
    # fp32's 4 (cost: reduced multiply precision; accumulation stays fp32).
    mmdt = {
        "fp32r": mybir.dt.float32r,
        "fp16": mybir.dt.float16,
    }[_IN_PREC]
    outdt = {"fp32": f32, "fp16": mybir.dt.float16}[_OUT_PREC]
    nc = bacc.Bacc("TRN2", target_bir_lowering=False, debug=False)
    xk = nc.dram_tensor("xk", [_NG, 128, _KTB * _TCORE], mmdt, kind="ExternalInput")
    cm = nc.dram_tensor("cm", [128, _NKT * 128], mmdt, kind="ExternalInput")
    yk = nc.dram_tensor("yk", [_NG, 128, _KTB * _TCORE], outdt, kind="ExternalOutput")

    with tile.TileContext(nc) as tc:
        with (
            tc.tile_pool(name="cpool", bufs=1) as cpool,
            tc.tile_pool(name="xpool", bufs=1) as xpool,
            tc.tile_pool(name="ypool", bufs=6) as ypool,
            tc.tile_pool(name="pp", bufs=3, space="PSUM") as pp,
        ):
            # cm in per-group chunks on the SP ring, interleaved with xk so
            # each group's dependencies land just in time
            cts = []
            xts = []
            for g in range(_NG):
                ct = cpool.tile([128, _KTB * 128], mmdt, tag=f"cw{g}", name=f"cw{g}")
                nc.sync.dma_start(
                    out=ct[:],
                    in_=cm[:, g * _KTB * 128:(g + 1) * _KTB * 128],
                )
                cts.append(ct)
                xt = xpool.tile(
                    [128, _KTB * _TCORE], mmdt, tag=f"x{g}", name=f"x{g}"
                )
                nc.sync.dma_start(out=xt[:], in_=xk[g])
                xts.append(xt)
            copy_idx = 0
            for g in range(_NG):
                xt = xts[g]
                yt = ypool.tile([128, _KTB * _TCORE], outdt)
                for h in range(_KTB // 2):
                    # 2-bank PSUM tile, two matmuls, one wide copy
                    ps = pp.tile([128, 2 * _TCORE], f32)
                    for jj in range(2):
                        j = h * 2 + jj
                        nc.tensor.matmul(
                            ps[:, jj * _TCORE:(jj + 1) * _TCORE],
                            lhsT=cts[g][:, j * 128:(j + 1) * 128],
                            rhs=xt[:, j * _TCORE:(j + 1) * _TCORE],
                            start=True,
                            stop=True,
                        )
                    yslice = yt[:, h * 2 * _TCORE:(h + 1) * 2 * _TCORE]
                    # every 3rd wide copy goes to ACT, rest to DVE
                    if copy_idx % 3 == 2:
                        nc.scalar.copy(yslice, ps[:])
                    else:
                        nc.vector.tensor_copy(yslice, ps[:])
                    copy_idx += 1
                # stores on the ACT HWDGE ring
                nc.scalar.dma_start(out=yk[g], in_=yt[:])
    nc.compile()
    return nc


def _execute(in_maps, **kwargs):
    from concourse.bass_utils import run_bass_kernel_spmd

    if "nc" not in _CACHE:
        _CACHE["nc"] = _build_device()
    return run_bass_kernel_spmd(
        _CACHE["nc"], in_maps, core_ids=list(range(_NCORES)), **kwargs
    )


def _make_in_maps(x, c):
    XKf = _build_xk(x)
    cmd = _build_cmat(c)
    maps = []
    for m in range(_NCORES):
        xkm = XKf[:, :, m * _TCORE:(m + 1) * _TCORE]  # [NKT, 128, TCORE]
        xkm = (
            xkm.reshape(_NG, _KTB, 128, _TCORE)
            .transpose(0, 2, 1, 3)
            .reshape(_NG, 128, _KTB * _TCORE)
        )
        maps.append(
            {"xk": np.ascontiguousarray(xkm).astype(_NP_IN[_IN_PREC]), "cm": cmd}
        )
    return maps


def _gather_yk(results):
    """Per-core yk [NG, 128, KTB*TCORE] -> full [NKT, 128, T]."""
    per_core = []
    for r in results:
        ykm = np.asarray(r["yk"]).reshape(_NG, 128, _KTB, _TCORE)
        per_core.append(ykm.transpose(0, 2, 1, 3).reshape(_NKT, 128, _TCORE))
    return np.concatenate(per_core, axis=2)


def kernel(x, c, bias, **_kwargs):
    in_maps = _make_in_maps(x, c)
    bkr = _execute(in_maps)
    return _unpack_y(_gather_yk(bkr.results), bias)


# revision 30
# speedup vs baseline: 1.0858x; 1.0683x over previous
"""Block-circulant linear layer (y = x @ W^T + bias, W built from 64x64
circulant blocks) on 8 Trainium2 NeuronCores.

Math: per output block j, input block i: y[t,j] = sum_i circ(c[j,i]) @ x[t,i].
Via the convolution theorem this is, for each rfft bin k:
    Yhat[t,j,k] = sum_i Chat[j,i,k] * Xhat[t,i,k]   (complex)
i.e. 33 independent complex [64 x 64] matmuls over the block index, batched
over tokens. The host does the cheap O(T*F*logB) DFTs + layout packing; the
device does the dominant compute — the per-frequency complex matmuls — packed
as 32 real [128x128] @ [128x512] matmuls per core (data-parallel over tokens).

Real/complex packing (per frequency k, contraction over rows r):
    rhs rows r:   [Xr_i (64) ; Xi_i (64)],  cols = tokens
    lhsT[i,    j] =  Cr[j,i]    lhsT[i,    64+j] = Ci[j,i]
    lhsT[64+i, j] = -Ci[j,i]    lhsT[64+i, 64+j] = Cr[j,i]
    out rows:     [Yr_j (64) ; Yi_j (64)]
Bins k=0 and k=32 are purely real (real input DFT), so they share one tile
(kt=0) with a block-diagonal lhsT; kt=1..31 carry bin k = kt.
"""

import numpy as np

_B = 64          # circulant block size
_NBLK = 64       # input/output blocks (4096/64)
_NK = 33         # rfft bins of a 64-point real signal
_NKT = 32        # packed frequency tiles (k0+k32 share tile 0)
_NCORES = 8
_T = 4096        # tokens = 2*2048
_TCORE = _T // _NCORES
_F = 4096

_CACHE = {}


# 4 k-tiles ride in each DMA transfer (512 KB in fp16): free-dim layout (kt%KTB, t)
_KTB = 4
_NG = _NKT // _KTB   # 8 DMA groups

# matmul input precision: "fp32r" (fp32 bits, TF32-grade multiply) or "fp16"
# (half the input DMA bytes, ~4x coarser rounding). Accumulation is fp32 either way.
_IN_PREC = "fp16"
# device->host precision of the frequency-domain result (fp16 halves store bytes;
# values are O(10) so fp16 rounding is ~2e-4 relative)
_OUT_PREC = "fp16"
_NP_IN = {"fp32r": np.float32, "fp16": np.float16}


def _build_cmat(c):
    """c: [J=64, I=64, B=64] float32 -> packed lhsT matrix [128, NKT*128]."""
    fc = np.fft.rfft(np.asarray(c, np.float32), axis=-1)  # [J, I, 33] complex64
    Cr, Ci = fc.real, fc.imag
    cm = np.zeros((_NKT, 128, 128), np.float32)  # [kt, row, col]
    cm[0, 0:64, 0:64] = Cr[:, :, 0].T
    cm[0, 64:128, 64:128] = Cr[:, :, 32].T
    for k in range(1, 32):
        cm[k, 0:64, 0:64] = Cr[:, :, k].T
        cm[k, 64:128, 0:64] = -Ci[:, :, k].T
        cm[k, 0:64, 64:128] = Ci[:, :, k].T
        cm[k, 64:128, 64:128] = Cr[:, :, k].T
    # device layout: [128 partitions, kt*128 + col]
    out = np.ascontiguousarray(cm.transpose(1, 0, 2)).reshape(128, _NKT * 128)
    return out.astype(_NP_IN[_IN_PREC])


def _build_xk(x):
    """x: [2, 2048, 4096] float32 -> packed rhs [NKT, 128, T]."""
    xb = np.asarray(x, np.float32).reshape(_T, _NBLK, _B)
    fx = np.fft.rfft(xb, axis=-1)            # [T, I, 33] complex64
    R = fx.real.transpose(2, 1, 0)           # [33, I, T]
    Im = fx.imag.transpose(2, 1, 0)
    XKf = np.empty((_NKT, 128, _T), np.float32)
    XKf[0, 0:64] = R[0]
    XKf[0, 64:128] = R[32]
    XKf[1:32, 0:64] = R[1:32]
    XKf[1:32, 64:128] = Im[1:32]
    return XKf


def _unpack_y(YKf, bias):
    """YKf: [NKT, 128, T] device output -> y [2, 2048, 4096] float32."""
    re = np.zeros((_NK, _NBLK, _T), np.float32)
    im = np.zeros((_NK, _NBLK, _T), np.float32)
    re[0] = YKf[0, 0:64]
    re[32] = YKf[0, 64:128]
    re[1:32] = YKf[1:32, 0:64]
    im[1:32] = YKf[1:32, 64:128]
    Yf = (re + 1j * im).transpose(2, 1, 0)   # [T, J, 33]
    yb = np.fft.irfft(Yf, n=_B, axis=-1).astype(np.float32)  # [T, J, B]
    y = yb.reshape(_T, _F) + np.asarray(bias, np.float32)
    return np.ascontiguousarray(y.reshape(2, _T // 2, _F))


def _build_device():
    import concourse.bacc as bacc
    import concourse.mybir as mybir
    import concourse.tile as tile

    f32 = mybir.dt.float32
    # float32r: same fp32 bits, but the PE streams 1 column/cycle instead of
    # fp32's 4 (cost: reduced multiply precision; accumulation stays fp32).
    mmdt = {
        "fp32r": mybir.dt.float32r,
        "fp16": mybir.dt.float16,
    }[_IN_PREC]
    outdt = {"fp32": f32, "fp16": mybir.dt.float16}[_OUT_PREC]
    nc = bacc.Bacc("TRN2", target_bir_lowering=False, debug=False)
    xk = nc.dram_tensor("xk", [_NG, 128, _KTB * _TCORE], mmdt, kind="ExternalInput")
    cm = nc.dram_tensor("cm", [128, _NKT * 128], mmdt, kind="ExternalInput")
    yk = nc.dram_tensor("yk", [_NG, 128, _KTB * _TCORE], outdt, kind="ExternalOutput")

    with tile.TileContext(nc) as tc:
        with (
            tc.tile_pool(name="cpool", bufs=1) as cpool,
            tc.tile_pool(name="xpool", bufs=1) as xpool,
            tc.tile_pool(name="ypool", bufs=6) as ypool,
            tc.tile_pool(name="pp", bufs=3, space="PSUM") as pp,
            tc.tile_pool(name="wpp", bufs=1, space="PSUM") as wpp,
        ):
            # cm in per-group chunks on the SP ring, interleaved with xk so
            # each group's dependencies land just in time
            cts = []
            xts = []
            for g in range(_NG):
                ct = cpool.tile([128, _KTB * 128], mmdt, tag=f"cw{g}", name=f"cw{g}")
                nc.sync.dma_start(
                    out=ct[:],
                    in_=cm[:, g * _KTB * 128:(g + 1) * _KTB * 128],
                )
                cts.append(ct)
                xt = xpool.tile(
                    [128, _KTB * _TCORE], mmdt, tag=f"x{g}", name=f"x{g}"
                )
                nc.sync.dma_start(out=xt[:], in_=xk[g])
                xts.append(xt)
            # HAM warmup: dummy matmuls on zeroed tiles while the first loads
            # are in flight, so the real matmul stream runs at 2.4 GHz
            # instead of the cold 1.2 GHz gate.
            wlhs = cpool.tile([128, 128], mmdt, tag="wlhs", name="wlhs")
            wrhs = cpool.tile([128, _TCORE], mmdt, tag="wrhs", name="wrhs")
            nc.gpsimd.memset(wlhs[:], 0.0)
            nc.gpsimd.memset(wrhs[:], 0.0)
            wps = wpp.tile([128, _TCORE], f32, name="wps")
            for _w in range(12):
                nc.tensor.matmul(
                    wps[:], lhsT=wlhs[:], rhs=wrhs[:], start=True, stop=True
                )
            copy_idx = 0
            for g in range(_NG):
                xt = xts[g]
                yt = ypool.tile([128, _KTB * _TCORE], outdt)
                for h in range(_KTB // 2):
                    # 2-bank PSUM tile, two matmuls, one wide copy
                    ps = pp.tile([128, 2 * _TCORE], f32)
                    for jj in range(2):
                        j = h * 2 + jj
                        nc.tensor.matmul(
                            ps[:, jj * _TCORE:(jj + 1) * _TCORE],
                            lhsT=cts[g][:, j * 128:(j + 1) * 128],
                            rhs=xt[:, j * _TCORE:(j + 1) * _TCORE],
                            start=True,
                            stop=True,
                        )
                    yslice = yt[:, h * 2 * _TCORE:(h + 1) * 2 * _TCORE]
                    # every 3rd wide copy goes to ACT, rest to DVE
                    if copy_idx % 3 == 2:
                        nc.scalar.copy(yslice, ps[:])
                    else:
                        nc.vector.tensor_copy(yslice, ps[:])
                    copy_idx += 1
                # stores on the ACT HWDGE ring
                nc.scalar.dma_start(out=yk[g], in_=yt[:])
    nc.compile()
    return nc


def _execute(in_maps, **kwargs):
    from concourse.bass_utils import run_bass_kernel_spmd

    if "nc" not in _CACHE:
        _CACHE["nc"] = _build_device()
    return run_bass_kernel_spmd(
        _CACHE["nc"], in_maps, core_ids=list(range(_NCORES)), **kwargs
    )


def _make_in_maps(x, c):
    XKf = _build_xk(x)
    cmd = _build_cmat(c)
    maps = []
    for m in range(_NCORES):
        xkm = XKf[:, :, m * _TCORE:(m + 1) * _TCORE]  # [NKT, 128, TCORE]
        xkm = (
            xkm.reshape(_NG, _KTB, 128, _TCORE)
            .transpose(0, 2, 1, 3)
            .reshape(_NG, 128, _KTB * _TCORE)
        )
        maps.append(
            {"xk": np.ascontiguousarray(xkm).astype(_NP_IN[_IN_PREC]), "cm": cmd}
        )
    return maps


def _gather_yk(results):
    """Per-core yk [NG, 128, KTB*TCORE] -> full [NKT, 128, T]."""
    per_core = []
    for r in results:
        ykm = np.asarray(r["yk"]).reshape(_NG, 128, _KTB, _TCORE)
        per_core.append(ykm.transpose(0, 2, 1, 3).reshape(_NKT, 128, _TCORE))
    return np.concatenate(per_core, axis=2)


def kernel(x, c, bias, **_kwargs):
    in_maps = _make_in_maps(x, c)
    bkr = _execute(in_maps)
    return _unpack_y(_gather_yk(bkr.results), bias)
